# revision 10
# baseline (speedup 1.0000x reference)
"""Trainium2 Bass kernel for nn_Model_17085379903564 (HiPPO-LegT multiscale
spectral forecaster).

Math: the reference normalizes x per (b,e) series, runs a HiPPO-LegT scan,
takes 32 rFFT modes of the state trajectory, mixes modes with complex
weights w, evaluates the irFFT at t=511, projects on Legendre polynomials
(Em), mixes two scales with an MLP, and un-normalizes.

Everything from the input to the Legendre projection is LINEAR with
constant coefficients, so per scale (L = 512 or 1024):

  Exf[be, (n,k)] = sum_t f[t,be] * W2[t,(n,k)]        (one dense operator)
  xdc[be,o]      = sum_(n,k) Re(Exf).Re(w) - Im(Exf).Im(w)
  dec            = xdc @ Em[-512:].T

where W2 folds the scan kernel G[m] = Ad^m Bd, the DFT, and the point-irFFT
weights e_k. W2 is numerically low rank (~240), so we factor it by SVD,
W2 ~= U @ V; stage 1 becomes g = f.T @ U, and the w-contraction collapses
through the factorization: P = V @ w is INDEPENDENT of x, and

  xdc_partial = g @ P        (per core, partial over its n-slice of V/w)

The instance norm commutes through all of it: with raw (un-normed) x,
  xdc_n = inv * (xdc_raw - mu x (SU @ P)),   SU = colsum(U)
and the final un-norm multiplies by std = 1/inv, so inv cancels:
  out = sum_s w_s * (std*xdc_n,s) @ EmT_s + (b*std + mu)
      = sum_s w_s * (xdc_raw,s - mu x (SU_s@P_s)) @ EmT_s + (b*std + mu)

Sharding (8 cores): V/w sharded over the spectral dim n (32 of 256 rows
per core) -> per-core partial dec; dec computed in transposed (p, be)
layout so one ReduceScatter hands each core its own 64-row slice of the
prediction horizon; only a rank-1 bias add + store remain after the
collective. x, U, Em and the mlp scalars are replicated.
"""

from contextlib import ExitStack

import ml_dtypes
import numpy as np

import concourse.bacc as bacc
import concourse.bass as bass
import concourse.mybir as mybir
import concourse.tile as tile
from concourse.bass_utils import run_bass_kernel_spmd
from concourse.masks import make_identity

# ---- problem constants (hardcoded; kernel.py must be self-contained) ----
B_SZ = 4
SEQ_LEN = 1024
PRED_LEN = 512
E_IN = 32
N_ORD = 256
MODES = 32
MULTISCALE = (1, 2)
BE = B_SZ * E_IN            # 128
N_CORES = 8
NSL = N_ORD // N_CORES      # 32  n-rows per core
NK = NSL * MODES            # 1024 stage-2 contraction length per core
PSL = PRED_LEN // N_CORES   # 64  output horizon slice per core
RANK = 256                  # SVD rank kept for the W2 operators

F32 = mybir.dt.float32
BF16 = mybir.dt.bfloat16
BF16_NP = np.dtype(ml_dtypes.bfloat16)


# ---------------------------------------------------------------- constants
def _transition_lmu(N):
    Q = np.arange(N, dtype=np.float64)
    R = (2 * Q + 1)[:, None]
    j, i = np.meshgrid(Q, Q)
    A = np.where(i < j, -1.0, (-1.0) ** (i - j + 1)) * R
    Bv = ((-1.0) ** Q[:, None] * R)[:, 0]
    return A, Bv


def _bilinear(A, Bv, dt):
    I = np.eye(A.shape[0])
    M = I - (dt / 2.0) * A
    Ad = np.linalg.solve(M, I + (dt / 2.0) * A)
    Bd = np.linalg.solve(M, dt * Bv)
    return Ad, Bd


def _legendre_vander(x, N):
    P = np.zeros((N, x.shape[0]))
    P[0] = 1.0
    if N > 1:
        P[1] = x
    for n in range(1, N - 1):
        P[n + 1] = ((2 * n + 1) * x * P[n] - n * P[n - 1]) / (n + 1)
    return P.T


def _scale_consts(ms):
    """Per-scale constants.

    Returns (U, Vre, Vim, SU, EmT):
      U   (L, RANK)            stage-1 left factor
      Vre (RANK, N_ORD*MODES)  right factor . real part of e_k-folded op
      Vim (RANK, N_ORD*MODES)  right factor . NEGATED imag part
      SU  (RANK,)              column sums of U (norm correction)
      EmT (N_ORD, PRED_LEN)    Em[-512:].T
    """
    L = ms * PRED_LEN
    A, Bv = _transition_lmu(N_ORD)
    Ad, Bd = _bilinear(A, Bv, 1.0 / L)
    vals = np.arange(0.0, 1.0, 1.0 / L)
    Em = _legendre_vander(1.0 - 2.0 * vals, N_ORD)        # (L, N)

    G = np.empty((L, N_ORD))
    g = Bd.copy()
    for m in range(L):
        G[m] = g
        g = Ad @ g
    k = np.arange(MODES)
    z = np.exp(-2j * np.pi * k / L)                       # (32,)
    zm = z[None, :] ** np.arange(L)[:, None]              # (L, 32)
    Gpre = np.cumsum(zm[:, None, :] * G[:, :, None], axis=0)   # (L, N, 32)
    W = zm[:, None, :] * Gpre[::-1]                       # (L, N, 32) complex
    e = (2.0 - (k == 0)) / L * np.exp(2j * np.pi * k * (PRED_LEN - 1) / L)
    W2 = W * e[None, None, :]

    M = np.concatenate(
        [W2.real.reshape(L, -1), (-W2.imag).reshape(L, -1)], axis=1)
    Uf, sv, Vt = np.linalg.svd(M, full_matrices=False)
    U = np.ascontiguousarray(Uf[:, :RANK])                # (L, r)
    V = sv[:RANK, None] * Vt[:RANK]                       # (r, 16384)
    Vre = np.ascontiguousarray(V[:, :N_ORD * MODES])
    Vim = np.ascontiguousarray(V[:, N_ORD * MODES:])
    SU = U.sum(axis=0)                                    # (r,)
    return U, Vre, Vim, SU, Em[-PRED_LEN:].T


_CONSTS = None


def _get_consts():
    global _CONSTS
    if _CONSTS is None:
        _CONSTS = [_scale_consts(ms) for ms in MULTISCALE]
    return _CONSTS


# ---------------------------------------------------------------- bass prog
def _build_nc():
    nc = bacc.Bacc("TRN2", target_bir_lowering=False, debug=False,
                   num_devices=N_CORES)

    p = {}
    p["xt"] = nc.declare_dram_parameter("xt", [BE, SEQ_LEN], F32,
                                        isOutput=False)
    p["ftx"] = nc.declare_dram_parameter("ftx", [SEQ_LEN, BE], BF16,
                                         isOutput=False)
    for s in (0, 1):
        L = (s + 1) * PRED_LEN
        p[f"u{s}"] = nc.declare_dram_parameter(f"u{s}", [L, RANK], BF16,
                                               isOutput=False)
        p[f"su{s}"] = nc.declare_dram_parameter(f"su{s}", [RANK, 1], BF16,
                                                isOutput=False)
        p[f"emt{s}"] = nc.declare_dram_parameter(f"emt{s}",
                                                 [N_ORD, PRED_LEN], BF16,
                                                 isOutput=False)
        for part in ("re", "im"):
            p[f"vt{part}{s}"] = nc.declare_dram_parameter(
                f"vt{part}{s}", [NK, RANK], BF16, isOutput=False)
            p[f"w{part}{s}"] = nc.declare_dram_parameter(
                f"w{part}{s}", [NK, N_ORD], BF16, isOutput=False)
    p["mlpw"] = nc.declare_dram_parameter("mlpw", [1, 2], F32,
                                          isOutput=False)
    p["mlpb"] = nc.declare_dram_parameter("mlpb", [1, 1], F32,
                                          isOutput=False)
    p["out_dec"] = nc.declare_dram_parameter("out_dec", [PSL, BE], F32,
                                             isOutput=True)

    with tile.TileContext(nc, num_cores=N_CORES) as tc:
        _emit(nc, tc, p)
    nc.finalize()
    return nc


def _emit(nc, tc, p):
    AF = mybir.ActivationFunctionType
    with ExitStack() as ctx:
        const = ctx.enter_context(tc.tile_pool(name="const", bufs=1))
        work = ctx.enter_context(tc.tile_pool(name="work", bufs=1))
        wpool = ctx.enter_context(tc.tile_pool(name="wts", bufs=2))
        ps_tr = ctx.enter_context(
            tc.tile_pool(name="ps_tr", bufs=2, space="PSUM"))
        ps_acc = ctx.enter_context(
            tc.tile_pool(name="ps_acc", bufs=2, space="PSUM"))
        ps_p = ctx.enter_context(
            tc.tile_pool(name="ps_p", bufs=2, space="PSUM"))
        ps_dec = ctx.enter_context(
            tc.tile_pool(name="ps_dec", bufs=2, space="PSUM"))
        dram = ctx.enter_context(
            tc.tile_pool(name="dram", bufs=1, space="DRAM"))

        ident = const.tile([128, 128], F32, tag="ident")
        make_identity(nc, ident[:])
        ident_b = const.tile([128, 128], BF16, tag="ident_b")
        make_identity(nc, ident_b[:])

        # ---- raw x (time-major) straight into stage-1 lhsT tiles --------
        ftx = const.tile([128, SEQ_LEN // 128, BE], BF16, tag="ftx")
        nc.sync.dma_start(ftx[:], p["ftx"].rearrange("(c p) f -> p c f",
                                                     p=128))

        # ---- series stats (off critical path; only mu/std consumed) -----
        xt_t = work.tile([BE, SEQ_LEN], F32, tag="xt")
        nc.scalar.dma_start(xt_t[:], p["xt"][:, :])
        sumx = work.tile([BE, 1], F32, tag="sumx")
        nc.vector.reduce_sum(sumx[:], xt_t[:], axis=mybir.AxisListType.X)
        sq = work.tile([BE, SEQ_LEN], F32, tag="sq")
        sumsq = work.tile([BE, 1], F32, tag="sumsq")
        nc.scalar.activation(sq[:], xt_t[:], AF.Square, accum_out=sumsq[:])
        mean = work.tile([BE, 1], F32, tag="mean")
        nc.scalar.mul(mean[:], sumx[:], 1.0 / SEQ_LEN)
        ex2 = work.tile([BE, 1], F32, tag="ex2")
        nc.scalar.mul(ex2[:], sumsq[:], 1.0 / SEQ_LEN)
        m2 = work.tile([BE, 1], F32, tag="m2")
        nc.scalar.square(m2[:], mean[:])
        var = work.tile([BE, 1], F32, tag="var")
        nc.vector.tensor_sub(var[:], ex2[:], m2[:])
        eps = work.tile([BE, 1], F32, tag="eps")
        nc.vector.memset(eps[:], 1e-5)
        std = work.tile([BE, 1], F32, tag="std")
        nc.scalar.activation(std[:], var[:], AF.Sqrt, bias=eps[:])

        # mu as a bf16 row vector (for the rank-1 norm correction)
        ps_mu = ps_tr.tile([1, 128], F32, tag="tr", name="ps_mu")
        nc.tensor.transpose(ps_mu[:], mean[:], ident[:])
        mu_row = work.tile([1, 128], BF16, tag="mu_row")
        nc.vector.tensor_copy(mu_row[:], ps_mu[:])

        # mlp scalar broadcasts: ws_sb[p, s] = mlp_weight[0, s] forall p
        mlpw_sb = const.tile([1, 2], F32, tag="mlpw")
        nc.sync.dma_start(mlpw_sb[:], p["mlpw"][:, :])
        mlpb_sb = const.tile([1, 1], F32, tag="mlpb")
        nc.sync.dma_start(mlpb_sb[:], p["mlpb"][:, :])
        ones = const.tile([1, 128], F32, tag="ones")
        nc.vector.memset(ones[:], 1.0)
        ps_w = ps_tr.tile([128, 2], F32, tag="tr", name="ps_w")
        nc.tensor.matmul(ps_w[:], lhsT=ones[:], rhs=mlpw_sb[:])
        ws_sb = work.tile([128, 2], F32, tag="ws")
        nc.vector.tensor_copy(ws_sb[:], ps_w[:])
        ps_b = ps_tr.tile([128, 1], F32, tag="tr", name="ps_b")
        nc.tensor.matmul(ps_b[:], lhsT=ones[:], rhs=mlpb_sb[:])
        bs_sb = work.tile([128, 1], F32, tag="bs")
        nc.vector.tensor_copy(bs_sb[:], ps_b[:])
        # bmu = mlp_bias*std + mean  (the post-collective bias row)
        bmu = work.tile([BE, 1], F32, tag="bmu")
        nc.vector.tensor_mul(bmu[:], bs_sb[:], std[:])
        nc.vector.tensor_add(bmu[:], bmu[:], mean[:])
        ps_bmu = ps_tr.tile([1, 128], F32, tag="tr", name="ps_bmu")
        nc.tensor.transpose(ps_bmu[:], bmu[:], ident[:])
        bmu_row = work.tile([1, 128], F32, tag="bmu_row")
        nc.vector.tensor_copy(bmu_row[:], ps_bmu[:])

        # ---- per scale: P = V@w (x-independent), g = f.T@U, xdc = g@P ---
        xdcT = {}
        for s in (0, 1):
            L = (s + 1) * PRED_LEN
            lch = L // 128
            j0 = SEQ_LEN // 128 - lch
            # stream in this scale's operands (split across both HWDGEs)
            vt, wt = {}, {}
            for part in ("re", "im"):
                vt[part] = wpool.tile([128, NK // 128, RANK], BF16,
                                      tag=f"vt{part}", name=f"vt{part}")
                nc.sync.dma_start(
                    vt[part][:],
                    p[f"vt{part}{s}"].rearrange("(c p) f -> p c f", p=128))
                wt[part] = wpool.tile([128, NK // 128, N_ORD], BF16,
                                      tag=f"wt{part}", name=f"wt{part}")
                nc.scalar.dma_start(
                    wt[part][:],
                    p[f"w{part}{s}"].rearrange("(c p) f -> p c f", p=128))
            u_t = wpool.tile([128, lch, RANK], BF16, tag="u", name="u_t")
            nc.scalar.dma_start(
                u_t[:], p[f"u{s}"].rearrange("(c p) f -> p c f", p=128))
            su_t = wpool.tile([128, 2, 1], BF16, tag="su", name="su_t")
            nc.sync.dma_start(
                su_t[:], p[f"su{s}"].rearrange("(c p) f -> p c f", p=128))

            # P[r, o] partial over this core's nk rows (two 128-row chunks)
            p_sb = []
            for rc in (0, 1):
                pps = ps_p.tile([128, N_ORD], F32, tag="pps", name="pps")
                nmm = NK // 128
                for i in range(nmm):
                    for part in ("re", "im"):
                        nc.tensor.matmul(
                            pps[:],
                            lhsT=vt[part][:, i, rc * 128:(rc + 1) * 128],
                            rhs=wt[part][:, i, :],
                            start=(i == 0 and part == "re"),
                            stop=(i == nmm - 1 and part == "im"))
                t = work.tile([128, N_ORD], BF16, tag=f"p{rc}",
                              name=f"p_sb{rc}")
                nc.vector.tensor_copy(t[:], pps[:])
                p_sb.append(t)

            # tp[o] = SU @ P  (1 x N_ORD), negated for the correction
            tp_ps = ps_tr.tile([1, N_ORD], F32, tag="tr", name="tp_ps")
            for rc in (0, 1):
                nc.tensor.matmul(tp_ps[:], lhsT=su_t[:, rc, :],
                                 rhs=p_sb[rc][:], start=(rc == 0),
                                 stop=(rc == 1))
            tp_neg = work.tile([1, N_ORD], BF16, tag="tp", name="tp_neg")
            nc.scalar.mul(tp_neg[:], tp_ps[:], -1.0)

            # g = f.T @ U   (raw x; norm correction is rank-1, added below)
            g_ps = ps_acc.tile([BE, RANK], F32, tag="acc", name="g_ps")
            for d in range(lch):
                nc.tensor.matmul(g_ps[:], lhsT=ftx[:, j0 + d, :],
                                 rhs=u_t[:, d, :],
                                 start=(d == 0), stop=(d == lch - 1))
            g_sb = work.tile([BE, RANK], BF16, tag="g", name="g_sb")
            nc.vector.tensor_copy(g_sb[:], g_ps[:])
            gT = []
            for rc in (0, 1):
                pst = ps_tr.tile([128, 128], BF16, tag="tr", name="pst")
                nc.tensor.transpose(
                    pst[:], g_sb[:, rc * 128:(rc + 1) * 128], ident_b[:])
                t = work.tile([128, BE], BF16, tag=f"gT{rc}",
                              name=f"gT{rc}")
                nc.vector.tensor_copy(t[:], pst[:])
                gT.append(t)

            # xdc_raw = g @ P - mu x tp   (rank-1 appended to the group)
            xdc_ps = ps_acc.tile([BE, N_ORD], F32, tag="acc",
                                 name="xdc_ps")
            for rc in (0, 1):
                nc.tensor.matmul(xdc_ps[:], lhsT=gT[rc][:],
                                 rhs=p_sb[rc][:],
                                 start=(rc == 0), stop=False)
            nc.tensor.matmul(xdc_ps[:], lhsT=mu_row[:], rhs=tp_neg[:],
                             start=False, stop=True)
            # scale by std (un-norm; inv cancelled algebraically)
            xdc_sb = work.tile([BE, N_ORD], F32, tag=f"xdc{s}",
                               name=f"xdc_sb{s}")
            nc.scalar.activation(xdc_sb[:], xdc_ps[:], AF.Copy,
                                 scale=std[:])
            # transpose to (o, be), scaling by mlp_weight[s] on the way
            for och in (0, 1):
                pst = ps_tr.tile([128, 128], F32, tag="tr", name="pst2")
                nc.tensor.transpose(
                    pst[:], xdc_sb[:, och * 128:(och + 1) * 128],
                    ident[:])
                t = work.tile([128, BE], BF16, tag=f"xdcT{s}{och}",
                              name=f"xdcT{s}{och}")
                nc.scalar.activation(t[:], pst[:], AF.Copy,
                                     scale=ws_sb[:, s:s + 1])
                xdcT[s, och] = t

        # ---- decT[p, be] = sum_s w_s (sigma xdc_s) @ EmT_s, transposed --
        emt_sb = []
        for s in (0, 1):
            t = const.tile([128, 2, PRED_LEN], BF16, tag=f"emt{s}",
                           name=f"emt_sb{s}")
            nc.scalar.dma_start(
                t[:], p[f"emt{s}"].rearrange("(c p) f -> p c f", p=128))
            emt_sb.append(t)

        dec_sb = work.tile([128, 4, BE], F32, tag="dec_sb")
        for pc in range(4):
            dps = ps_dec.tile([128, BE], F32, tag="dec", name="dps")
            first = True
            for s in (0, 1):
                for och in (0, 1):
                    nc.tensor.matmul(
                        dps[:],
                        lhsT=emt_sb[s][:, och, pc * 128:(pc + 1) * 128],
                        rhs=xdcT[s, och][:],
                        start=first, stop=(s == 1 and och == 1))
                    first = False
            nc.vector.tensor_copy(dec_sb[:, pc, :], dps[:])

        # ---- ReduceScatter partial decT; rank-1 bias; store -------------
        bounce_in = dram.tile([4 * 128, BE], F32, tag="bin")
        bounce_out = dram.tile([PSL, BE], F32, tag="bout")
        nc.gpsimd.dma_start(
            bounce_in.rearrange("(c p) f -> p c f", p=128), dec_sb[:])
        nc.gpsimd.collective_compute(
            "ReduceScatter",
            mybir.AluOpType.add,
            replica_groups=[list(range(N_CORES))],
            ins=[bounce_in.opt()],
            outs=[bounce_out.opt()],
        )
        dd = work.tile([PSL, BE], F32, tag="dd")
        nc.gpsimd.dma_start(dd[:], bounce_out[:])

        ones64 = const.tile([1, PSL], F32, tag="ones64")
        nc.vector.memset(ones64[:], 1.0)
        corr_ps = ps_tr.tile([PSL, 128], F32, tag="tr", name="corr_ps")
        nc.tensor.matmul(corr_ps[:], lhsT=ones64[:], rhs=bmu_row[:])
        out_sb = work.tile([PSL, BE], F32, tag="out")
        nc.vector.tensor_add(out_sb[:], dd[:], corr_ps[:])
        nc.sync.dma_start(p["out_dec"][:, :], out_sb[:])


_NC = None


def _get_nc():
    global _NC
    if _NC is None:
        _NC = _build_nc()
    return _NC


# ---------------------------------------------------------------- host side
_CONST_MAPS = None


def _const_maps():
    global _CONST_MAPS
    if _CONST_MAPS is None:
        consts = _get_consts()
        _CONST_MAPS = []
        for c in range(N_CORES):
            n0 = c * NSL
            m = {}
            for s in (0, 1):
                U, Vre, Vim, SU, EmT = consts[s]
                m[f"u{s}"] = np.ascontiguousarray(U).astype(BF16_NP)
                m[f"su{s}"] = np.ascontiguousarray(
                    SU.reshape(RANK, 1)).astype(BF16_NP)
                m[f"emt{s}"] = np.ascontiguousarray(EmT).astype(BF16_NP)
                for part, V in (("re", Vre), ("im", Vim)):
                    vs = V.reshape(RANK, N_ORD, MODES)[:, n0:n0 + NSL, :]
                    m[f"vt{part}{s}"] = np.ascontiguousarray(
                        vs.reshape(RANK, NK).T).astype(BF16_NP)
            _CONST_MAPS.append(m)
    return _CONST_MAPS


def _in_maps(x_enc, spec_w_real, spec_w_imag, mlp_weight, mlp_bias):
    xt = np.ascontiguousarray(
        np.transpose(x_enc, (0, 2, 1)).reshape(BE, SEQ_LEN)).astype(
            np.float32, copy=False)
    ftx = np.ascontiguousarray(
        x_enc.transpose(1, 0, 2).reshape(SEQ_LEN, BE)).astype(BF16_NP)
    mw = np.asarray(mlp_weight, np.float32).reshape(1, 2)
    mb = np.asarray(mlp_bias, np.float32).reshape(1, 1)
    shared = {"xt": xt, "ftx": ftx, "mlpw": mw, "mlpb": mb}

    maps = []
    for c in range(N_CORES):
        n0 = c * NSL
        m = dict(shared)
        m.update(_const_maps()[c])
        for s in (0, 1):
            m[f"wre{s}"] = np.ascontiguousarray(
                spec_w_real[s, n0:n0 + NSL].transpose(0, 2, 1).reshape(
                    NK, N_ORD)).astype(BF16_NP)
            m[f"wim{s}"] = np.ascontiguousarray(
                spec_w_imag[s, n0:n0 + NSL].transpose(0, 2, 1).reshape(
                    NK, N_ORD)).astype(BF16_NP)
        maps.append(m)
    return maps


def kernel(x_enc, spec_w_real, spec_w_imag, mlp_weight, mlp_bias,
           _trace=False, _trace_kwargs=None):
    x_enc = np.asarray(x_enc, np.float32)
    spec_w_real = np.asarray(spec_w_real, np.float32)
    spec_w_imag = np.asarray(spec_w_imag, np.float32)
    maps = _in_maps(x_enc, spec_w_real, spec_w_imag, mlp_weight, mlp_bias)
    nc = _get_nc()
    res = run_bass_kernel_spmd(nc, maps, list(range(N_CORES)),
                               trace=_trace, **(_trace_kwargs or {}))
    # out_dec[c] = decT rows [64c, 64c+64) x (be) -> out[b, p, e]
    full = np.concatenate(
        [res.results[c]["out_dec"] for c in range(N_CORES)], axis=0)
    out = full.reshape(PRED_LEN, B_SZ, E_IN).transpose(1, 0, 2)
    out = np.ascontiguousarray(out, np.float32)
    if _trace:
        return out, res
    return out
